# revision 8
# baseline (speedup 1.0000x reference)
"""Trainium2 Bass kernel for nn_MultiHeadAttention_75754633167392.

Multi-head attention with a dynamic per-query conv1d over keys
(per-head kernel widths KWS = [1,1,1,1,3,3,5,7], zero-padded to 7 taps).

Sharding: pure data-parallel over batch — B == n_cores == 8, one batch
element per NeuronCore, no collectives.

Per-core algorithm (all fp32):
  - host pre-transposes q/k/v to (D, L) so contraction over D feeds the
    PE naturally; all weights are host-packed into lhsT layouts.
  - attention logits S[i,j] = bias_q[i] + bias_b
        + sum_t (q_s @ WkerT_t + bker_t)[i,:] . k_pad[j+t,:]
    are evaluated as matmuls over a stacked contraction axis: pairs of
    adjacent taps (t, t+1) are stacked into 128-row operands, using a
    key buffer kTD whose lower 64 partitions hold k_sT and whose upper
    64 partitions hold k_sT shifted by one key position.  The per-query
    bias rides along as one extra contraction row (lhs row = bias_tot[i],
    rhs row = 1).  Only nonzero taps are computed.
  - S is computed in BOTH orientations on the PE (i-major for the attn
    output + softmax, j-major for the attn @ v contraction); operands are
    identical, only stationary/moving roles swap.
  - softmax skips max-subtraction (logit rowmax <= ~15, exp is safe in
    fp32; softmax is shift-invariant).
  - ctx^T is accumulated with v_aug (v_s with an appended ones column) so
    the softmax denominators fall out of the same matmul as row 64.
  - per-head 64-row operands for even/odd heads live in the lower/upper
    64 partitions of shared tiles; matmuls address them via row/col
    tile_position offsets.
"""

import numpy as np

import concourse.bass as bass
import concourse.bacc as bacc
import concourse.mybir as mybir
import concourse.tile as tile
from concourse.bass_utils import run_bass_kernel_spmd

F32 = mybir.dt.float32
LAST_RES = None

B, L, D, H, DK, KW, PAD = 8, 1024, 512, 8, 64, 7, 3
KWS = [1, 1, 1, 1, 3, 3, 5, 7]
TEMPER = float(DK) ** 0.5
NCORES = 8
NIB = L // 128            # 8 row blocks of 128
NIC = L // 512            # 2 column chunks of 512
NKB = D // 128            # 4 contraction blocks of 128


def _head_taps(h):
    kw = KWS[h]
    off = (KW - kw) // 2
    return list(range(off, off + kw))


def _head_blocks(h):
    """Split the head's nonzero taps into adjacent pairs + one tail tap."""
    taps = _head_taps(h)
    pairs = []
    while len(taps) > 1:
        pairs.append((taps[0], taps[1]))
        taps = taps[2:]
    return pairs, taps[0]


# Wker tiles are packed per head-PAIR: tile (p, t) holds WkerT[2p, t] in
# rows 0-63 and WkerT[2p+1, t] in rows 64-127 (zeros where the head lacks
# the tap).
_WKER_IDX = {}
for _p in range(H // 2):
    for _t in sorted(set(_head_taps(2 * _p)) | set(_head_taps(2 * _p + 1))):
        _WKER_IDX[(_p, _t)] = len(_WKER_IDX)
N_WKER = len(_WKER_IDX)   # 12

_BKER_IDX = {}
for _h in range(H):
    _pairs, _tail = _head_blocks(_h)
    for _pi, _ in enumerate(_pairs):
        _BKER_IDX[(_h, _pi)] = len(_BKER_IDX)
    _BKER_IDX[(_h, "tail")] = len(_BKER_IDX)
N_BKER = len(_BKER_IDX)   # 15

# cols_pack column layout: bq_pair (4) | Wqb_pair (4) | bk_dup (8) | bker (15)
_COL_BQ = 0
_COL_WQB = 4
_COL_BK = 8
_COL_BKER = 16
N_COLS = 16 + N_BKER


def build_program(cb):
    """Build the single-core Bass program. cb[h] = bqb[h] + bias_b[h]."""
    nc = bacc.Bacc(None, target_bir_lowering=False)

    qT_d = nc.dram_tensor("qT", [D, L], F32, kind="ExternalInput")
    kT_d = nc.dram_tensor("kT", [D, L], F32, kind="ExternalInput")
    vT_d = nc.dram_tensor("vT", [D, L], F32, kind="ExternalInput")
    WqT_d = nc.dram_tensor("WqT", [D, D], F32, kind="ExternalInput")
    Wkd_d = nc.dram_tensor("Wk_dup", [H, 128, D], F32, kind="ExternalInput")
    WvT_d = nc.dram_tensor("WvT", [D, D], F32, kind="ExternalInput")
    WpT_d = nc.dram_tensor("WprojT", [D, D], F32, kind="ExternalInput")
    Wker_d = nc.dram_tensor("Wker_lhsT", [N_WKER, 128, DK], F32, kind="ExternalInput")
    cols_d = nc.dram_tensor("cols_pack", [128, N_COLS], F32, kind="ExternalInput")
    bv_d = nc.dram_tensor("bv_bc", [128, D], F32, kind="ExternalInput")
    bp_d = nc.dram_tensor("bproj_bc", [128, D], F32, kind="ExternalInput")

    out_d = nc.dram_tensor("out", [L, D], F32, kind="ExternalOutput")
    attn_d = nc.dram_tensor("attn", [H, L, L], F32, kind="ExternalOutput")
    oha_d = nc.dram_tensor("oha", [L, L], F32, kind="ExternalOutput")

    inv_temper = 1.0 / TEMPER

    with tile.TileContext(nc) as tc:
        with (
            tc.tile_pool(name="persist", bufs=1) as pp,
            tc.tile_pool(name="ktd", bufs=2) as ktd_pool,
            tc.tile_pool(name="wkd", bufs=2) as wkd_pool,
            tc.tile_pool(name="stack", bufs=1) as stack_pool,
            tc.tile_pool(name="work", bufs=2) as work,
            tc.tile_pool(name="psS", bufs=2, space="PSUM") as psS,
            tc.tile_pool(name="psctx", bufs=1, space="PSUM") as psctx,
            tc.tile_pool(name="pssmall", bufs=2, space="PSUM") as pssm,
        ):
            # ---------- persistent operands ----------
            kT = [pp.tile([128, L], F32, name=f"kT{i}") for i in range(NKB)]
            for i in range(NKB):
                nc.sync.dma_start(kT[i][:], kT_d[128 * i:128 * (i + 1), :])
            Wker = {}
            for (p, t), idx in _WKER_IDX.items():
                w = pp.tile([128, DK], F32, name=f"Wker{idx}")
                nc.sync.dma_start(w[:], Wker_d[idx, :, :])
                Wker[(p, t)] = w
            cols = pp.tile([128, N_COLS], F32, name="cols")
            nc.sync.dma_start(cols[:], cols_d[:])
            bv_bc = pp.tile([128, D], F32, name="bv_bc")
            nc.sync.dma_start(bv_bc[:], bv_d[:])
            bp_bc = pp.tile([128, D], F32, name="bp_bc")
            nc.sync.dma_start(bp_bc[:], bp_d[:])
            WpT = [pp.tile([128, D], F32, name=f"WpT{i}") for i in range(NKB)]
            for i in range(NKB):
                nc.sync.dma_start(WpT[i][:], WpT_d[128 * i:128 * (i + 1), :])
            ones_col = pp.tile([1, 64], F32, name="ones_col")
            nc.vector.memset(ones_col[:], 1.0)

            v_aug = [pp.tile([128, 65 * H], F32, name=f"vaug{j}") for j in range(NIB)]
            q_sT = [pp.tile([128, L], F32, name=f"qsT{p}") for p in range(H // 2)]
            ctxT = [pp.tile([128, L], F32, name=f"ctxT{i}") for i in range(NKB)]

            # ---------- prep phase (transient tiles): q_sT, then v_aug ----------
            with tc.tile_pool(name="prepq", bufs=1) as prep:
                qT = [prep.tile([128, L], F32, name=f"qT{i}") for i in range(NKB)]
                WqT = [prep.tile([128, D], F32, name=f"WqT{i}") for i in range(NKB)]
                for i in range(NKB):
                    nc.sync.dma_start(qT[i][:], qT_d[128 * i:128 * (i + 1), :])
                    nc.sync.dma_start(WqT[i][:], WqT_d[128 * i:128 * (i + 1), :])

                # q projection: head h -> rows 64*(h%2) of q_sT[h//2]
                for h in range(H):
                    r = 64 * (h % 2)
                    for ic in range(NIC):
                        pq = pssm.tile([128, 512], F32, name="ps_small",
                                       tag="ps_small")
                        for kb in range(NKB):
                            nc.tensor.matmul(
                                pq[r:r + 64, :], WqT[kb][:, 64 * h:64 * (h + 1)],
                                qT[kb][:, 512 * ic:512 * (ic + 1)],
                                start=(kb == 0), stop=(kb == NKB - 1),
                                tile_position=(0, r))
                        nc.vector.tensor_scalar_add(
                            q_sT[h // 2][r:r + 64, 512 * ic:512 * (ic + 1)],
                            pq[r:r + 64, :],
                            cols[r:r + 64, _COL_BQ + h // 2:_COL_BQ + h // 2 + 1])

            with tc.tile_pool(name="prepv", bufs=1) as prep:
                vT = [prep.tile([128, L], F32, name=f"vT{i}") for i in range(NKB)]
                WvT = [prep.tile([128, D], F32, name=f"WvT{i}") for i in range(NKB)]
                for i in range(NKB):
                    nc.sync.dma_start(vT[i][:], vT_d[128 * i:128 * (i + 1), :])
                    nc.sync.dma_start(WvT[i][:], WvT_d[128 * i:128 * (i + 1), :])

                # v projection -> v_aug (interleaved [64 v | 1 ones] per head)
                for j in range(NIB):
                    pv = pssm.tile([128, D], F32, name="ps_small", tag="ps_small")
                    for kb in range(NKB):
                        nc.tensor.matmul(
                            pv[:], vT[kb][:, 128 * j:128 * (j + 1)], WvT[kb][:],
                            start=(kb == 0), stop=(kb == NKB - 1))
                    for h in range(H):
                        nc.vector.tensor_tensor(
                            out=v_aug[j][:, 65 * h:65 * h + 64],
                            in0=pv[:, 64 * h:64 * (h + 1)],
                            in1=bv_bc[:, 64 * h:64 * (h + 1)],
                            op=mybir.AluOpType.add)
                    ones_v = v_aug[j].rearrange("p (h c) -> p h c", c=65)[:, :, 64:65]
                    nc.vector.memset(ones_v, 1.0)

            # ---------- head loop ----------
            for h in range(H):
                p, r = h // 2, 64 * (h % 2)
                pairs, tail = _head_blocks(h)
                qs = q_sT[p]

                # --- k projection (duplicated rows) into shifted key buffers ---
                # kTD rows 0-63:  k_sT[h][c, u] at columns u = j + 3   (k_pad index)
                # kTD rows 64-127: same shifted by one key position
                # kTD2: rows 0-63 as kTD, row 64 = ones (bias lane), rows 65+ = 0
                wkd = wkd_pool.tile([128, D], F32, name="wkd", tag="wkd")
                nc.sync.dma_start(wkd[:], Wkd_d[h, :, :])
                ktd2 = ktd_pool.tile([128, L + 8], F32, name="ktd2", tag="ktd2")
                nc.gpsimd.memset(ktd2[:], 0.0)
                nc.vector.memset(ktd2[64:65, :], 1.0)
                ktd = None
                if pairs:
                    ktd = ktd_pool.tile([128, L + 8], F32, name="ktd", tag="ktd")
                    nc.gpsimd.memset(ktd[:], 0.0)
                bk_col = cols[:, _COL_BK + h:_COL_BK + h + 1]
                for ic in range(NIC):
                    pk = pssm.tile([128, 512], F32, name="ps_small", tag="ps_small")
                    for kb in range(NKB):
                        nc.tensor.matmul(
                            pk[:], wkd[:, 128 * kb:128 * (kb + 1)],
                            kT[kb][:, 512 * ic:512 * (ic + 1)],
                            start=(kb == 0), stop=(kb == NKB - 1))
                    nc.vector.tensor_scalar_add(
                        ktd2[0:64, 3 + 512 * ic:3 + 512 * ic + 512],
                        pk[0:64, :], bk_col[0:64, :])
                    if pairs:
                        nc.vector.tensor_scalar_add(
                            ktd[0:64, 3 + 512 * ic:3 + 512 * ic + 512],
                            pk[0:64, :], bk_col[0:64, :])
                        nc.vector.tensor_scalar_add(
                            ktd[64:128, 2 + 512 * ic:2 + 512 * ic + 512],
                            pk[64:128, :], bk_col[64:128, :])

                # --- Kt stacks over the stacked contraction axis ---
                # pair pi: rows 0-63 = q_s @ WkerT_tA + bker_tA; rows 64-127 = tap tB
                # tail:    rows 0-63 = tap t_tail; row 64 = bias_tot[i]; rows 65+ = 0
                stacks = []  # (sbuf_tile, key_buffer, top_tap)
                for pi, (tA, tB) in enumerate(pairs):
                    st = stack_pool.tile([128, L], F32, name=f"stack{pi}",
                                         tag=f"stack{pi}")
                    bcol = cols[:, _COL_BKER + _BKER_IDX[(h, pi)]:
                                _COL_BKER + _BKER_IDX[(h, pi)] + 1]
                    for ic in range(NIC):
                        pkt = pssm.tile([128, 512], F32, name="ps_small",
                                        tag="ps_small")
                        rhs = qs[r:r + 64, 512 * ic:512 * (ic + 1)]
                        nc.tensor.matmul(pkt[0:64, :], Wker[(p, tA)][r:r + 64, :],
                                         rhs, start=True, stop=True,
                                         tile_position=(r, 0))
                        nc.tensor.matmul(pkt[64:128, :], Wker[(p, tB)][r:r + 64, :],
                                         rhs, start=True, stop=True,
                                         tile_position=(r, 64))
                        nc.vector.tensor_scalar_add(
                            st[:, 512 * ic:512 * (ic + 1)], pkt[:], bcol)
                    stacks.append((st, ktd, tA))

                st_tail = stack_pool.tile([128, L], F32, name="stack_tail",
                                          tag="stack_tail")
                bcol_t = cols[:, _COL_BKER + _BKER_IDX[(h, "tail")]:
                              _COL_BKER + _BKER_IDX[(h, "tail")] + 1]
                nc.vector.memset(st_tail[64:128, :], 0.0)
                for ic in range(NIC):
                    pkt = pssm.tile([128, 512], F32, name="ps_small", tag="ps_small")
                    rhs = qs[r:r + 64, 512 * ic:512 * (ic + 1)]
                    nc.tensor.matmul(pkt[0:64, :], Wker[(p, tail)][r:r + 64, :],
                                     rhs, start=True, stop=True,
                                     tile_position=(r, 0))
                    nc.tensor.matmul(
                        pkt[64:65, :],
                        cols[r:r + 64, _COL_WQB + p:_COL_WQB + p + 1],
                        rhs, start=True, stop=True, tile_position=(r, 64))
                    nc.vector.tensor_scalar_add(
                        st_tail[0:64, 512 * ic:512 * (ic + 1)], pkt[0:64, :],
                        bcol_t[0:64, :])
                    nc.vector.tensor_scalar_add(
                        st_tail[64:65, 512 * ic:512 * (ic + 1)], pkt[64:65, :],
                        float(cb[h]))
                stacks.append((st_tail, ktd2, tail))

                # --- natural orientation: S[i, j] -> softmax -> attn output ---
                for ib in range(NIB):
                    pS = psS.tile([128, L], F32, name="psS", tag="psS")
                    for jc in range(NIC):
                        for bi, (st, kbuf, tA) in enumerate(stacks):
                            nc.tensor.matmul(
                                pS[:, 512 * jc:512 * (jc + 1)],
                                st[:, 128 * ib:128 * (ib + 1)],
                                kbuf[:, 512 * jc + tA:512 * jc + tA + 512],
                                start=(bi == 0), stop=(bi == len(stacks) - 1))
                    expn = work.tile([128, L], F32, name="expn", tag="expn")
                    sums = work.tile([128, 1], F32, name="sums", tag="sums")
                    nc.scalar.activation(expn[:], pS[:],
                                         mybir.ActivationFunctionType.Exp,
                                         scale=inv_temper, accum_out=sums[:])
                    rcp = work.tile([128, 1], F32, name="rcp", tag="rcp")
                    nc.vector.reciprocal(rcp[:], sums[:])
                    attn_t = work.tile([128, L], F32, name="attn_t", tag="attn_t",
                                       bufs=3)
                    nc.vector.tensor_scalar_mul(attn_t[:], expn[:], rcp[:])
                    nc.sync.dma_start(
                        attn_d[h, 128 * ib:128 * (ib + 1), :], attn_t[:])
                    if h == 0:
                        nc.sync.dma_start(
                            oha_d[128 * ib:128 * (ib + 1), :], attn_t[:])

                # --- transposed orientation: S_T[j, i] -> exp -> ctx ---
                pctx = psctx.tile([65, L], F32, name="psctx", tag="psctx")
                for jb in range(NIB):
                    pT = psS.tile([128, L], F32, name="psS", tag="psS")
                    for ic in range(NIC):
                        for bi, (st, kbuf, tA) in enumerate(stacks):
                            nc.tensor.matmul(
                                pT[:, 512 * ic:512 * (ic + 1)],
                                kbuf[:, 128 * jb + tA:128 * jb + tA + 128],
                                st[:, 512 * ic:512 * (ic + 1)],
                                start=(bi == 0), stop=(bi == len(stacks) - 1))
                    expT = work.tile([128, L], F32, name="expT", tag="expT")
                    nc.scalar.activation(expT[:], pT[:],
                                         mybir.ActivationFunctionType.Exp,
                                         scale=inv_temper)
                    for ic in range(NIC):
                        nc.tensor.matmul(
                            pctx[:, 512 * ic:512 * (ic + 1)],
                            v_aug[jb][:, 65 * h:65 * (h + 1)],
                            expT[:, 512 * ic:512 * (ic + 1)],
                            start=(jb == 0), stop=(jb == NIB - 1))

                # --- normalize ctx^T rows by the softmax denominators ---
                rcp_row = work.tile([1, L], F32, name="rcp_row", tag="rcp_row")
                nc.vector.reciprocal(rcp_row[:], pctx[64:65, :])
                bc_sb = work.tile([64, L], F32, name="bc_sb", tag="bc_sb")
                for ic in range(NIC):
                    pbc = pssm.tile([128, 512], F32, name="ps_small", tag="ps_small")
                    nc.tensor.matmul(pbc[0:64, :], ones_col[:],
                                     rcp_row[:, 512 * ic:512 * (ic + 1)],
                                     start=True, stop=True)
                    nc.vector.tensor_copy(bc_sb[:, 512 * ic:512 * (ic + 1)],
                                          pbc[0:64, :])
                nc.vector.tensor_tensor(
                    out=ctxT[p][r:r + 64, :],
                    in0=pctx[0:64, :], in1=bc_sb[:],
                    op=mybir.AluOpType.mult)

            # ---------- output projection ----------
            for ib in range(NIB):
                po = pssm.tile([128, 512], F32, name="ps_small", tag="ps_small")
                for kb in range(NKB):
                    nc.tensor.matmul(
                        po[:], ctxT[kb][:, 128 * ib:128 * (ib + 1)], WpT[kb][:],
                        start=(kb == 0), stop=(kb == NKB - 1))
                out_t = work.tile([128, D], F32, name="out_t", tag="out_t")
                nc.vector.tensor_tensor(out=out_t[:], in0=po[:], in1=bp_bc[:],
                                        op=mybir.AluOpType.add)
                nc.sync.dma_start(out_d[128 * ib:128 * (ib + 1), :], out_t[:])

    nc.compile()
    return nc


def _prep_weights(Wq, bq, Wk, bk, Wv, bv, Wker, bker, Wqb, bqb, bias_b,
                  Wproj, bproj):
    w = {}
    w["WqT"] = np.ascontiguousarray(Wq.T)
    WkT = Wk.T
    # Wk_dup[h]: (128, D) lhsT rows; row c = Wk[h*64+c], rows 64-127 duplicate.
    # Stored as (H, 128, D): [h, c, kd] = WkT[kd, h*64 + (c % 64)] transposed
    # into per-kblk lhsT layout: lhsT block kb = Wk_dup[h][:, kb*128:...]^T ...
    # We need lhsT[k=kd_block, m] per kb as tile[:, kb*128:+128] with partition
    # = kd?  NO — tile is (128 partitions = dup'd head rows? ...
    wkd = np.zeros((H, 128, D), np.float32)
    for h in range(H):
        blk = WkT[:, 64 * h:64 * (h + 1)]           # (D, 64): [kd, c]
        dup = np.concatenate([blk, blk], axis=1)    # (D, 128): [kd, m]
        # wkd[h][:, kb*128:+128] is lhsT for block kb: [kd_within (partition), m]
        for kb in range(NKB):
            wkd[h, :, 128 * kb:128 * (kb + 1)] = dup[128 * kb:128 * (kb + 1), :]
    w["Wk_dup"] = wkd
    w["WvT"] = np.ascontiguousarray(Wv.T)
    w["WprojT"] = np.ascontiguousarray(Wproj.T)
    wker = np.zeros((N_WKER, 128, DK), np.float32)
    for (p, t), idx in _WKER_IDX.items():
        for s in range(2):
            h = 2 * p + s
            if t in _head_taps(h):
                wker[idx, 64 * s:64 * (s + 1), :] = Wker[h, :, t, :].T
    w["Wker_lhsT"] = wker
    cols = np.zeros((128, N_COLS), np.float32)
    for h in range(H):
        p, r = h // 2, 64 * (h % 2)
        cols[r:r + 64, _COL_BQ + p] = bq[64 * h:64 * (h + 1)]
        cols[r:r + 64, _COL_WQB + p] = Wqb[h]
        cols[:, _COL_BK + h] = np.concatenate([bk[64 * h:64 * (h + 1)]] * 2)
        pairs, tail = _head_blocks(h)
        for pi, (tA, tB) in enumerate(pairs):
            cols[0:64, _COL_BKER + _BKER_IDX[(h, pi)]] = bker[h, :, tA]
            cols[64:128, _COL_BKER + _BKER_IDX[(h, pi)]] = bker[h, :, tB]
        cols[0:64, _COL_BKER + _BKER_IDX[(h, "tail")]] = bker[h, :, tail]
    w["cols_pack"] = cols
    w["bv_bc"] = np.ascontiguousarray(np.broadcast_to(bv, (128, D)))
    w["bproj_bc"] = np.ascontiguousarray(np.broadcast_to(bproj, (128, D)))
    cb = [float(bqb[h] + bias_b[h]) for h in range(H)]
    return w, cb


def kernel(q, k, v, attn_mask, Wq, bq, Wk, bk, Wv, bv,
           Wker, bker, Wqb, bqb, bias_b, Wproj, bproj):
    del attn_mask  # all-False per the input spec; where(mask, -inf, .) is a no-op
    arrs = {n: np.asarray(a, np.float32) for n, a in dict(
        q=q, k=k, v=v, Wq=Wq, bq=bq, Wk=Wk, bk=bk, Wv=Wv, bv=bv, Wker=Wker,
        bker=bker, Wqb=Wqb, bqb=bqb, bias_b=bias_b, Wproj=Wproj,
        bproj=bproj).items()}

    w, cb = _prep_weights(
        arrs["Wq"], arrs["bq"], arrs["Wk"], arrs["bk"], arrs["Wv"], arrs["bv"],
        arrs["Wker"], arrs["bker"], arrs["Wqb"], arrs["bqb"], arrs["bias_b"],
        arrs["Wproj"], arrs["bproj"])

    nc = build_program(cb)

    in_maps = []
    for b in range(NCORES):
        m = dict(w)
        m["qT"] = np.ascontiguousarray(arrs["q"][b].T)
        m["kT"] = np.ascontiguousarray(arrs["k"][b].T)
        m["vT"] = np.ascontiguousarray(arrs["v"][b].T)
        in_maps.append(m)

    global LAST_RES
    res = run_bass_kernel_spmd(nc, in_maps, list(range(NCORES)))
    LAST_RES = res
    results = res.results

    out = np.stack([results[b]["out"] for b in range(NCORES)])
    attn = np.stack([results[b]["attn"] for b in range(NCORES)])
    oha = np.stack([results[b]["oha"] for b in range(NCORES)])
    return out, attn, oha
